# revision 1
# baseline (speedup 1.0000x reference)
"""TRN2 Bass kernel for nn_ClassSemantic (scatter_memory).

Strategy (v2)
-------------
Data-parallel over batch: core k owns samples 4k..4k+3 and runs
projection (1x1 conv) + memory-gather attention + concat on device.

The kernel is memory-regime: the fp32 baseline was at ~95% of the DMA
roofline (67 MB/core).  v2 cuts HBM bytes ~2.3x:
  - feats are sent as bf16 (host converts; fp16 is NOT used because the
    PE streams fp16 moving operands at 2 cycles/col vs bf16's 1)
  - the x output half is written as bf16 (~4e-3 rel err vs 2e-2 gate)
  - the attention output half u is written as fp8-e4m3 (|u| <= 0.08
    while the output scale is 5.7, so fp8 error is ~1e-4 of scale)
Host packs feats into the exact per-chunk SBUF layout so each chunk is
ONE contiguous 512KB DMA descriptor (descriptor issue costs ~0.7us of
engine time each; baseline used 5/chunk, v2 uses 3/chunk).

With bytes halved the PE becomes the critical engine (~84us); the
attention path is arranged to minimize PE column passes:
  - logits per chunk land in [20, NT] PSUM at base partition 0; the
    exp ACTIVATE partition-shifts its write into a persistent
    [128, NT] tile at offset 32*(c%3), batching 3 chunks per group
    (engines are column-bound, so ops on [20, NT] cost the same as
    [128, NT]; matmuls may NOT write PSUM at partition offsets, but
    engine writes and matmul READS at offsets 0/32/64 are legal)
  - the softmax column-sum for a whole group is ONE [128,128]
    block-diagonal ones matmul (rows outside the 20 valid slots of
    each 32-row strip are zeroed by a one-time zeros DMA)
  - recip + attn-normalize run once per group on [128, NT]
  - the u matmuls contract the 20-row strip at offset 32*(c%3)

The sequential EMA queue update depends only on per-sample masked
feature means, computed on host (0.4% of FLOPs), and the final queue
rows are shipped to every core as constants; no collective needed.
"""
import os
import numpy as np
import ml_dtypes
from contextlib import ExitStack

B, IN_C, H, W_SP = 32, 512, 64, 64
CODE, CLASSES, MEM = 256, 4, 20
HW = H * W_SP              # 4096
NCORES = 8
BPC = B // NCORES          # 4 samples per core
DECAY, EPS = 0.9, 1e-12
NCH = 8                    # chunks per sample
NT = HW // NCH             # 512 spatial positions per chunk
T = BPC * NCH              # 32 chunks per core
GSZ = 3                    # chunks per softmax group (offsets 0/32/64)
NG = (T + GSZ - 1) // GSZ
F8 = ml_dtypes.float8_e4m3

_PROGRAM_CACHE = {}
LAST_RESULTS = None        # stash for test harness introspection


def _host_queue_update(feats, preds, labels, flag, queue, Wp, bp):
    """Final queue after the reference's sequential EMA scan (float64)."""
    if int(flag) != 1:
        return queue.astype(np.float32)
    f3 = feats.reshape(B, IN_C, HW)
    p2 = preds.reshape(B, HW)
    g = np.matmul(f3, p2[:, :, None])[:, :, 0] / np.float32(HW)
    feat = g @ Wp.T.astype(np.float32) + bp[None, :] * p2.mean(axis=1)[:, None]
    q = queue.astype(np.float64)
    for i in range(B):
        l = int(labels[i])
        f = feat[i].astype(np.float64)
        slot = q[l]
        logit = slot @ f
        upd = logit[:, None] * f[None, :]
        nrm = np.sqrt((upd * upd).sum(axis=1, keepdims=True))
        upd = upd / np.maximum(nrm, EPS)
        q[l] = DECAY * slot + (1.0 - DECAY) * upd
    return q.astype(np.float32)


def _build_program():
    from concourse import bacc, mybir
    import concourse.tile as tile

    f32, f32r = mybir.dt.float32, mybir.dt.float32r
    f16, f8 = mybir.dt.bfloat16, mybir.dt.float8e4
    nc = bacc.Bacc("TRN2", target_bir_lowering=False, debug=False)

    feats_in = nc.dram_tensor("feats", [T, 128, 4, NT], f16, kind="ExternalInput").ap()
    wpt_in = nc.dram_tensor("wpt", [128, 4, CODE], f16, kind="ExternalInput").ap()
    bp_in = nc.dram_tensor("bpc", [128, 2], f32, kind="ExternalInput").ap()
    qat_in = nc.dram_tensor("qat", [128, BPC, 2, MEM], f16, kind="ExternalInput").ap()
    qa4_in = nc.dram_tensor("qa4", [128, BPC, CODE], f16, kind="ExternalInput").ap()
    onesblk_in = nc.dram_tensor("onesblk", [128, 128], f32r, kind="ExternalInput").ap()
    zeros_in = nc.dram_tensor("zeros", [128, NT], f32r, kind="ExternalInput").ap()
    out_x = nc.dram_tensor("out_x", [T, 128, 2, NT], f16, kind="ExternalOutput").ap()
    out_u = nc.dram_tensor("out_u", [T, 128, 2, NT], f8, kind="ExternalOutput").ap()

    with tile.TileContext(nc) as tc, ExitStack() as ctx:
        consts = ctx.enter_context(tc.tile_pool(name="consts", bufs=1))
        fpool = ctx.enter_context(tc.tile_pool(name="fpool", bufs=4))
        xpool = ctx.enter_context(tc.tile_pool(name="xpool", bufs=6))
        rpool = ctx.enter_context(tc.tile_pool(name="rpool", bufs=2))
        npool = ctx.enter_context(tc.tile_pool(name="npool", bufs=2))
        upool = ctx.enter_context(tc.tile_pool(name="upool", bufs=4))
        ppp = ctx.enter_context(tc.tile_pool(name="ppp", bufs=3, space="PSUM"))
        plg = ctx.enter_context(tc.tile_pool(name="plg", bufs=2, space="PSUM"))
        pcs = ctx.enter_context(tc.tile_pool(name="pcs", bufs=1, space="PSUM"))
        ppu = ctx.enter_context(tc.tile_pool(name="ppu", bufs=2, space="PSUM"))

        wpt_sb = consts.tile([128, 4, CODE], f16, name="wpt_sb")
        bp_sb = consts.tile([128, 2], f32, name="bp_sb")
        qat_sb = consts.tile([128, BPC, 2, MEM], f16, name="qat_sb")
        qa4_sb = consts.tile([128, BPC, CODE], f16, name="qa4_sb")
        onesblk_sb = consts.tile([128, 128], f32r, name="onesblk_sb")
        # two persistent pexp buffers, zero-initialized once so the rows
        # outside the written 20-row strips stay 0 for the block matmul
        pexp_ab = [consts.tile([128, NT], f32r, name=f"pexp{i}") for i in range(2)]
        # constants ride the scalar ring so the sync ring starts on feats
        nc.scalar.dma_start(wpt_sb[:], wpt_in[:])
        nc.scalar.dma_start(bp_sb[:], bp_in[:])
        nc.scalar.dma_start(qat_sb[:], qat_in[:])
        nc.scalar.dma_start(qa4_sb[:], qa4_in[:])
        nc.scalar.dma_start(onesblk_sb[:], onesblk_in[:])
        nc.scalar.dma_start(pexp_ab[0][:], zeros_in[:])
        nc.scalar.dma_start(pexp_ab[1][:], zeros_in[:])

        x_t = {}
        lg_t = {}
        cs_t = {}
        pn_t = {}
        u_t = {}

        def bj(c):
            return c // NCH, c % NCH

        def glast(g):
            return min(GSZ * g + GSZ - 1, T - 1)

        def proj_chunk(c):
            ft = fpool.tile([128, 4, NT], f16, tag="ft", name=f"ft{c}")
            nc.sync.dma_start(ft[:], feats_in[c])
            xt = xpool.tile([128, 2, NT], f16, tag="xt", name=f"xt{c}")
            x_t[c] = xt
            for h in range(2):
                ps = ppp.tile([128, NT], f32, tag="proj_ps", name=f"pps{c}_{h}")
                for kk in range(4):
                    nc.tensor.matmul(
                        ps[:], wpt_sb[:, kk, h * 128:(h + 1) * 128], ft[:, kk, :],
                        start=(kk == 0), stop=(kk == 3))
                if h == 0:
                    nc.scalar.activation(
                        xt[:, 0, :], ps[:],
                        mybir.ActivationFunctionType.Identity,
                        bias=bp_sb[:, 0:1])
                else:
                    nc.vector.tensor_scalar_add(xt[:, 1, :], ps[:], bp_sb[:, 1:2])

        def x_flush(c):
            nc.gpsimd.dma_start(out_x[c], x_t[c][:])

        def logit_stage(c):
            b, _ = bj(c)
            lg = plg.tile([MEM, NT], f32, tag="lg", name=f"lg{c}")
            lg_t[c] = lg
            xt = x_t[c]
            for kk in range(2):
                nc.tensor.matmul(lg[:], qat_sb[:, b, kk, :], xt[:, kk, :],
                                 start=(kk == 0), stop=(kk == 1))

        def exp_stage(c):
            g, r = c // GSZ, c % GSZ
            pexp = pexp_ab[g % 2]
            nc.scalar.activation(pexp[32 * r:32 * r + MEM, :], lg_t.pop(c)[:],
                                 mybir.ActivationFunctionType.Exp)

        def colsum_group(g):
            cs = pcs.tile([128, NT], f32, tag="cs", name=f"cs{g}")
            nc.tensor.matmul(cs[:], onesblk_sb[:], pexp_ab[g % 2][:],
                             start=True, stop=True)
            cs_t[g] = cs

        def recip_pn_group(g):
            rc = rpool.tile([128, NT], f32, tag="rc", name=f"rc{g}")
            nc.vector.reciprocal_approx_fast(out=rc[:], in_=cs_t.pop(g)[:])
            pn = npool.tile([128, NT], f16, tag="pn", name=f"pn{g}")
            nc.gpsimd.tensor_mul(pn[:], pexp_ab[g % 2][:], rc[:])
            pn_t[g] = pn

        def u_stage(c):
            b, _ = bj(c)
            g, r = c // GSZ, c % GSZ
            pn = pn_t[g]
            s = slice(32 * r, 32 * r + MEM)
            ut = upool.tile([128, 2, NT], f8, tag="ut", name=f"ut{c}")
            u_t[c] = ut
            for h in range(2):
                us = ppu.tile([128, NT], f32, tag="u_ps", name=f"us{c}_{h}")
                nc.tensor.matmul(us[:], qa4_sb[s, b, h * 128:(h + 1) * 128],
                                 pn[s, :], start=True, stop=True)
                if h == 0:
                    nc.scalar.copy(ut[:, 0, :], us[:])
                else:
                    nc.vector.tensor_copy(ut[:, 1, :], us[:])

        def u_flush(c):
            nc.gpsimd.dma_start(out_u[c], u_t.pop(c)[:])

        # Chunk-level software pipeline; the group stages fire a couple
        # of iterations after the last member chunk's exp is emitted.
        colsum_at = {glast(g) + 4: g for g in range(NG)}
        recip_at = {glast(g) + 5: g for g in range(NG)}
        for t in range(T + 9):
            if t < T:
                proj_chunk(t)
            if 1 <= t and t - 1 < T:
                x_flush(t - 1)
            if 2 <= t and t - 2 < T:
                logit_stage(t - 2)
            if 3 <= t and t - 3 < T:
                exp_stage(t - 3)
            if t in colsum_at:
                colsum_group(colsum_at[t])
            if t in recip_at:
                recip_pn_group(recip_at[t])
            if 7 <= t and t - 7 < T:
                u_stage(t - 7)
            if 8 <= t and t - 8 < T:
                u_flush(t - 8)

    nc.compile()
    return nc


def kernel(feats, preds, labels, flag, queue, Wp, bp):
    from concourse.bass_utils import run_bass_kernel_spmd
    global LAST_RESULTS

    feats = np.ascontiguousarray(np.asarray(feats, dtype=np.float32))
    preds = np.ascontiguousarray(np.asarray(preds, dtype=np.float32))
    labels = np.asarray(labels).astype(np.int64)
    queue = np.ascontiguousarray(np.asarray(queue, dtype=np.float32))
    Wp = np.ascontiguousarray(np.asarray(Wp, dtype=np.float32))
    bp = np.ascontiguousarray(np.asarray(bp, dtype=np.float32))
    try:
        flag_v = int(np.asarray(flag))
    except TypeError:
        flag_v = int(flag)

    qfin = _host_queue_update(feats, preds, labels, flag_v, queue, Wp, bp)
    qA = qfin[labels]                                            # [B, 20, 256]
    # qat: [128, b, kk, m] with code c = kk*128 + p
    qat = np.ascontiguousarray(
        qA.transpose(0, 2, 1).reshape(B, 2, 128, MEM).transpose(2, 0, 1, 3)
        .astype(ml_dtypes.bfloat16))                             # [128, B, 2, 20]
    # qa4: [128, b, c] replicated at partition offsets 0/32/64(/96)
    qa4 = np.zeros((4, 32, B, CODE), dtype=ml_dtypes.bfloat16)
    qa4[:, :MEM] = qA.transpose(1, 0, 2)[None].astype(ml_dtypes.bfloat16)
    qa4 = qa4.reshape(128, B, CODE)
    wpt = np.ascontiguousarray(
        Wp.T.reshape(4, 128, CODE).transpose(1, 0, 2).astype(ml_dtypes.bfloat16))
    bpc = np.ascontiguousarray(bp.reshape(2, 128).T)
    # block-diagonal ones: 1 where row strip == col strip (32-row blocks)
    blk = np.arange(128) // 32
    onesblk = (blk[:, None] == blk[None, :]).astype(np.float32)
    zeros = np.zeros((128, NT), dtype=np.float32)

    # feats: [b, c, hw] -> chunk-major [t=b*8+j, p, kk, n], c = kk*128+p
    f16 = (feats.reshape(B, 4, 128, NCH, NT).transpose(0, 3, 2, 1, 4)
           .astype(ml_dtypes.bfloat16))                          # [B, 8, 128, 4, NT]

    if "prog" not in _PROGRAM_CACHE:
        _PROGRAM_CACHE["prog"] = _build_program()
    nc = _PROGRAM_CACHE["prog"]

    in_maps = []
    for k in range(NCORES):
        s = slice(k * BPC, (k + 1) * BPC)
        in_maps.append({
            "feats": np.ascontiguousarray(f16[s]).reshape(T, 128, 4, NT),
            "wpt": wpt,
            "bpc": bpc,
            "qat": np.ascontiguousarray(qat[:, s]),
            "qa4": np.ascontiguousarray(qa4[:, s]),
            "onesblk": onesblk,
            "zeros": zeros,
        })

    trace = bool(int(os.environ.get("KERNEL_TRACE", "0")))
    tc_env = os.environ.get("KERNEL_TRACE_CORES", "")
    trace_cores = [int(x) for x in tc_env.split(",") if x] or None
    res = run_bass_kernel_spmd(nc, in_maps, core_ids=list(range(NCORES)),
                               trace=trace, trace_cores=trace_cores)
    LAST_RESULTS = res

    out = np.empty((B, 2 * CODE, HW), dtype=np.float32)
    for k in range(NCORES):
        xk = res.results[k]["out_x"]          # [T, 128, 2, NT] bf16
        uk = res.results[k]["out_u"]          # [T, 128, 2, NT] fp8
        xk = (xk.reshape(BPC, NCH, 128, 2, NT).transpose(0, 3, 2, 1, 4)
              .reshape(BPC, CODE, HW).astype(np.float32))
        uk = (uk.astype(np.float32).reshape(BPC, NCH, 128, 2, NT)
              .transpose(0, 3, 2, 1, 4).reshape(BPC, CODE, HW))
        s = slice(k * BPC, (k + 1) * BPC)
        out[s, CODE:] = xk
        out[s, :CODE] = uk
    return out.reshape(B, 2 * CODE, H, W_SP)


if __name__ == "__main__":
    d = np.load("/tmp/inputs.npz")
    out = kernel(d["feats"], d["preds"], d["labels"], d["flag"], d["queue"], d["Wp"], d["bp"])
    exp = np.load("/tmp/expected.npy")
    err = np.abs(out - exp)
    print("absmax err:", err.max(), "scale-rel:", err.max() / np.abs(exp).max())



# revision 5
# speedup vs baseline: 1.1762x; 1.1762x over previous
"""TRN2 Bass kernel for nn_ClassSemantic (scatter_memory).

Strategy (v4)
-------------
Data-parallel over batch: core k owns samples 4k..4k+3 and runs
projection (1x1 conv) + memory-gather attention + concat on device.
The sequential EMA queue update depends only on per-sample masked
feature means, computed on host (0.4% of FLOPs); final queue rows are
shipped to every core as constants — no collective needed.

v4 vs v3 (128us):
  - the projection bias moves OFF the device x-path entirely: wpt is
    host-multiplied by 16, the evacuation is a plain one-op copy of the
    double-bank [128,2,NT] PSUM tile, and the host adds the bias after
    int8 dequant.  The logit path's missing q.b term is folded into the
    exp ACTIVATE's per-partition bias AP (exact, per group).  This
    halves the Scalar/Vector evacuation op count (their busy% nearly
    rivalled the PE in v3 and stretched the tail).
  - the 8 logit matmuls of a group are emitted as one kk-major
    interleaved batch so consecutive MMs target different array column
    groups and actually overlap (v3 emitted them per-chunk, separated
    by 8 projection MMs, so col tiling never overlapped anything)
  - u matmuls run in 4 two-MM row-tiled waves on 2 PSUM banks, one wave
    per step, so bank-reuse never stalls the PE FIFO
  - 8 warmup matmuls on a memset zeros tile pre-warm the HAM clock gate
  - outputs: x as int8 (x16, SWDGE cast-during-DMA, sign-aware host
    dequant), u as fp8-e4m3; both flushed in group-sized batches
"""
import os
import numpy as np
import ml_dtypes
from contextlib import ExitStack

B, IN_C, H, W_SP = 32, 512, 64, 64
CODE, CLASSES, MEM = 256, 4, 20
HW = H * W_SP              # 4096
NCORES = 8
BPC = B // NCORES          # 4 samples per core
DECAY, EPS = 0.9, 1e-12
NCH = 8                    # chunks per sample
NT = HW // NCH             # 512 spatial positions per chunk
T = BPC * NCH              # 32 chunks per core
GSZ = 4                    # chunks per group (strips at 0/32/64/96)
NG = T // GSZ              # 8 groups per core
XSC = 16.0                 # x-half int8 scale (power of 2: bf16-exact)
NWARM = 8                  # PE warmup matmuls
F8 = ml_dtypes.float8_e4m3

_PROGRAM_CACHE = {}
LAST_RESULTS = None        # stash for test harness introspection


def _host_queue_update(feats, preds, labels, flag, queue, Wp, bp):
    """Final queue after the reference's sequential EMA scan (float64)."""
    if int(flag) != 1:
        return queue.astype(np.float32)
    f3 = feats.reshape(B, IN_C, HW)
    p2 = preds.reshape(B, HW)
    g = np.matmul(f3, p2[:, :, None])[:, :, 0] / np.float32(HW)
    feat = g @ Wp.T.astype(np.float32) + bp[None, :] * p2.mean(axis=1)[:, None]
    q = queue.astype(np.float64)
    for i in range(B):
        l = int(labels[i])
        f = feat[i].astype(np.float64)
        slot = q[l]
        logit = slot @ f
        upd = logit[:, None] * f[None, :]
        nrm = np.sqrt((upd * upd).sum(axis=1, keepdims=True))
        upd = upd / np.maximum(nrm, EPS)
        q[l] = DECAY * slot + (1.0 - DECAY) * upd
    return q.astype(np.float32)


def _build_program():
    from concourse import bacc, mybir
    import concourse.tile as tile

    f32 = mybir.dt.float32
    f16, f8, i8 = mybir.dt.bfloat16, mybir.dt.float8e4, mybir.dt.int8
    nc = bacc.Bacc("TRN2", target_bir_lowering=False, debug=False)

    feats_in = nc.dram_tensor("feats", [T, 128, 4, NT], f16, kind="ExternalInput").ap()
    wpt_in = nc.dram_tensor("wpt", [128, 4, CODE], f16, kind="ExternalInput").ap()
    qat_in = nc.dram_tensor("qat", [128, BPC, 2, MEM], f16, kind="ExternalInput").ap()
    qa4_in = nc.dram_tensor("qa4", [128, BPC, CODE], f16, kind="ExternalInput").ap()
    ones_in = nc.dram_tensor("onesm", [128, 128], f16, kind="ExternalInput").ap()
    eoff_in = nc.dram_tensor("eoff", [128, NG], f32, kind="ExternalInput").ap()
    out_x = nc.dram_tensor("out_x", [128, T, 2, NT], i8, kind="ExternalOutput").ap()
    out_u = nc.dram_tensor("out_u", [128, T, 2, NT], f8, kind="ExternalOutput").ap()

    with tile.TileContext(nc) as tc, ExitStack() as ctx:
        consts = ctx.enter_context(tc.tile_pool(name="consts", bufs=1))
        fpool = ctx.enter_context(tc.tile_pool(name="fpool", bufs=6))
        xst = ctx.enter_context(tc.tile_pool(name="xst", bufs=3))
        ust = ctx.enter_context(tc.tile_pool(name="ust", bufs=2))
        epool = ctx.enter_context(tc.tile_pool(name="epool", bufs=2))
        rpool = ctx.enter_context(tc.tile_pool(name="rpool", bufs=2))
        npool = ctx.enter_context(tc.tile_pool(name="npool", bufs=2))
        ppp = ctx.enter_context(tc.tile_pool(name="ppp", bufs=2, space="PSUM"))
        plg = ctx.enter_context(tc.tile_pool(name="plg", bufs=1, space="PSUM"))
        pcs = ctx.enter_context(tc.tile_pool(name="pcs", bufs=1, space="PSUM"))
        ppu = ctx.enter_context(tc.tile_pool(name="ppu", bufs=2, space="PSUM"))

        wpt_sb = consts.tile([128, 4, CODE], f16, name="wpt_sb")
        qat_sb = consts.tile([128, BPC, 2, MEM], f16, name="qat_sb")
        qa4_sb = consts.tile([128, BPC, CODE], f16, name="qa4_sb")
        ones_sb = consts.tile([128, 128], f16, name="ones_sb")
        eoff_sb = consts.tile([128, NG], f32, name="eoff_sb")
        zeros_sb = consts.tile([128, NT], f16, name="zeros_sb")
        # persistent packed-logits PSUM bank: zero-initialized by warmup
        # matmuls; junk strips (p%32 >= 20) stay 0 forever
        lgP = plg.tile([128, NT], f32, name="lgP")

        # warmup: memset zeros (no DMA dependency), then accumulate
        # 0.T @ 0 into lgP to keep the PE busy from t~0 so the HAM clock
        # gate reaches 8/8 before the first feats chunk lands.  Doubles
        # as the one-time zero init of the logits bank.
        nc.gpsimd.memset(zeros_sb[:], 0.0)
        for i in range(NWARM):
            nc.tensor.matmul(lgP[:], zeros_sb[:, 0:128], zeros_sb[:],
                             start=(i == 0), stop=(i == NWARM - 1))

        # constants ride the scalar ring so the sync ring starts on feats
        nc.scalar.dma_start(wpt_sb[:], wpt_in[:])
        nc.scalar.dma_start(qat_sb[:], qat_in[:])
        nc.scalar.dma_start(qa4_sb[:], qa4_in[:])
        nc.scalar.dma_start(ones_sb[:], ones_in[:])
        nc.scalar.dma_start(eoff_sb[:], eoff_in[:])

        xst_t = {}
        ust_t = {}
        pexp_t = {}
        rc_t = {}
        pn_t = {}

        def proj_chunk(c):
            g, r = c // GSZ, c % GSZ
            if r == 0:
                xst_t[g] = xst.tile([128, GSZ, 2, NT], f16, tag="xs", name=f"xs{g}")
            ft = fpool.tile([128, 4, NT], f16, tag="ft", name=f"ft{c}")
            nc.sync.dma_start(ft[:], feats_in[c])
            xs = xst_t[g]
            pp = ppp.tile([128, 2, NT], f32, tag="proj_ps", name=f"pps{c}")
            for h in range(2):
                for kk in range(4):
                    nc.tensor.matmul(
                        pp[:, h, :], wpt_sb[:, kk, h * 128:(h + 1) * 128],
                        ft[:, kk, :], start=(kk == 0), stop=(kk == 3))
            # one-op evacuation of both halves (wpt carries the x16
            # scale; bias is applied on the host after dequant)
            if c % 2 == 0:
                nc.scalar.copy(xs[:, r, :, :], pp[:, :, :])
            else:
                nc.vector.tensor_copy(xs[:, r, :, :], pp[:, :, :])

        def logit_group(g):
            # kk-major interleave: consecutive MMs hit different column
            # groups of the array and overlap (col tiling)
            xs = xst_t[g]
            for kk in range(2):
                for r in range(GSZ):
                    b = (GSZ * g + r) // NCH
                    nc.tensor.matmul(lgP[32 * r:32 * r + MEM, :],
                                     qat_sb[:, b, kk, :], xs[:, r, kk, :],
                                     start=(kk == 0), stop=(kk == 1),
                                     tile_position=(0, 32 * r))

        def x_flush(g):
            # SWDGE cast bf16 -> int8 during DMA (truncates toward zero)
            nc.gpsimd.dma_start(out_x[:, GSZ * g:GSZ * (g + 1)], xst_t.pop(g)[:])

        def exp_group(g):
            # bias restores the q.b logit term dropped from the x path
            pexp = epool.tile([128, NT], f16, tag="pexp", name=f"pexp{g}")
            pexp_t[g] = pexp
            nc.scalar.activation(pexp[:], lgP[:],
                                 mybir.ActivationFunctionType.Exp,
                                 bias=eoff_sb[:, g:g + 1])

        def colsum_group(g):
            cs = pcs.tile([128, NT], f32, tag="cs", name=f"cs{g}")
            nc.tensor.matmul(cs[:], ones_sb[:], pexp_t[g][:],
                             start=True, stop=True)
            rc = rpool.tile([128, NT], f32, tag="rc", name=f"rc{g}")
            nc.vector.reciprocal_approx_fast(out=rc[:], in_=cs[:])
            rc_t[g] = rc

        def pn_group(g):
            pn = npool.tile([128, NT], f16, tag="pn", name=f"pn{g}")
            nc.gpsimd.tensor_mul(pn[:], pexp_t.pop(g)[:], rc_t.pop(g)[:])
            pn_t[g] = pn

        def u_wave(g, w):
            # wave w: chunks (2*(w%2), 2*(w%2)+1) x half w//2, two
            # row-tiled MMs on the 2 ppu banks; one wave per step so the
            # bank-reuse WAR never stalls the PE FIFO
            h, pr = w // 2, (w % 2) * 2
            pn = pn_t[g]
            if w == 0:
                ust_t[g] = ust.tile([128, GSZ, 2, NT], f8, tag="us", name=f"us{g}")
            us = ust_t[g]
            pst = []
            for r in (pr, pr + 1):
                b = (GSZ * g + r) // NCH
                s = slice(32 * r, 32 * r + MEM)
                up = ppu.tile([128, NT], f32, tag="u_ps", name=f"ups{g}_{h}_{r}")
                nc.tensor.matmul(up[:], qa4_sb[s, b, h * 128:(h + 1) * 128],
                                 pn[s, :], start=True, stop=True,
                                 tile_position=(32 * r, 0))
                pst.append((r, up))
            for i, (r, up) in enumerate(pst):
                if (r + h) % 2 == 0:
                    nc.scalar.copy(us[:, r, h, :], up[:])
                else:
                    nc.vector.tensor_copy(us[:, r, h, :], up[:])
            if w == 3:
                pn_t.pop(g)

        def u_flush(g):
            # fp8 -> fp8, HWDGE on the scalar (ACT) ring
            nc.scalar.dma_start(out_u[:, GSZ * g:GSZ * (g + 1)], ust_t.pop(g)[:])

        def glast(g):
            return GSZ * g + GSZ - 1

        lg_at = {glast(g) + 2: g for g in range(NG)}
        xf_at = {glast(g) + 2: g for g in range(NG)}
        exp_at = {glast(g) + 3: g for g in range(NG)}
        cs_at = {glast(g) + 4: g for g in range(NG)}
        pn_at = {glast(g) + 5: g for g in range(NG)}
        uw_at = {glast(g) + 6 + w: (g, w) for g in range(NG) for w in range(4)}
        uf_at = {glast(g) + 10: g for g in range(NG)}
        for t in range(T + 13):
            # exp(g) MUST precede any same-step logit emission: both
            # touch the shared lgP bank and the tile dep tracker
            # serializes them in program order.
            if t in exp_at:
                exp_group(exp_at[t])
            if t < T:
                proj_chunk(t)
            if t in lg_at:
                logit_group(lg_at[t])
            if t in xf_at:
                x_flush(xf_at[t])
            if t in cs_at:
                colsum_group(cs_at[t])
            if t in pn_at:
                pn_group(pn_at[t])
            if t in uw_at:
                u_wave(*uw_at[t])
            if t in uf_at:
                u_flush(uf_at[t])

    nc.compile()
    return nc


def kernel(feats, preds, labels, flag, queue, Wp, bp):
    from concourse.bass_utils import run_bass_kernel_spmd
    global LAST_RESULTS

    feats = np.ascontiguousarray(np.asarray(feats, dtype=np.float32))
    preds = np.ascontiguousarray(np.asarray(preds, dtype=np.float32))
    labels = np.asarray(labels).astype(np.int64)
    queue = np.ascontiguousarray(np.asarray(queue, dtype=np.float32))
    Wp = np.ascontiguousarray(np.asarray(Wp, dtype=np.float32))
    bp = np.ascontiguousarray(np.asarray(bp, dtype=np.float32))
    try:
        flag_v = int(np.asarray(flag))
    except TypeError:
        flag_v = int(flag)

    qfin = _host_queue_update(feats, preds, labels, flag_v, queue, Wp, bp)
    qA = qfin[labels]                                            # [B, 20, 256]
    # qat: [128, b, kk, m] with code c = kk*128 + p; pre-divided by XSC
    # so logits against the XSC-scaled bias-free x' are exact
    qat = np.ascontiguousarray(
        (qA / XSC).transpose(0, 2, 1).reshape(B, 2, 128, MEM).transpose(2, 0, 1, 3)
        .astype(ml_dtypes.bfloat16))                             # [128, B, 2, 20]
    # qa4: [128, b, c] replicated at partition offsets 0/32/64/96
    qa4 = np.zeros((4, 32, B, CODE), dtype=ml_dtypes.bfloat16)
    qa4[:, :MEM] = qA.transpose(1, 0, 2)[None].astype(ml_dtypes.bfloat16)
    qa4 = qa4.reshape(128, B, CODE)
    # wpt carries the XSC scale so the device x-path needs no scale op
    wpt = np.ascontiguousarray(
        (Wp.T * XSC).reshape(4, 128, CODE).transpose(1, 0, 2)
        .astype(ml_dtypes.bfloat16))
    # per-(group, partition) exp bias: the q.b logit term dropped from
    # the device x path.  partition p of the packed logits bank = strip
    # r=p//32 (chunk 4g+r -> sample) x slot m=p%32.
    qb = qA.astype(np.float64) @ bp.astype(np.float64)           # [B, 20]
    eoff = np.zeros((128, NG), dtype=np.float32)
    for g in range(NG):
        for r in range(GSZ):
            bsm = (GSZ * g + r) // NCH
            eoff[32 * r:32 * r + MEM, g] = qb[np.arange(BPC) == 0, :][0] * 0  # placeholder
    # (filled per-core below; eoff depends on the core's sample slice)
    # masked block-diagonal ones: 1 where row strip == col strip AND the
    # row is one of the 20 valid memory slots of its 32-strip
    blk = np.arange(128) // 32
    onesm = ((blk[:, None] == blk[None, :]) &
             ((np.arange(128) % 32) < MEM)[:, None]).astype(ml_dtypes.bfloat16)

    # feats: [b, c, hw] -> chunk-major [t=b*8+j, p, kk, n], c = kk*128+p
    f16 = (feats.reshape(B, 4, 128, NCH, NT).transpose(0, 3, 2, 1, 4)
           .astype(ml_dtypes.bfloat16))                          # [B, 8, 128, 4, NT]

    if "prog" not in _PROGRAM_CACHE:
        _PROGRAM_CACHE["prog"] = _build_program()
    nc = _PROGRAM_CACHE["prog"]

    in_maps = []
    for k in range(NCORES):
        s = slice(k * BPC, (k + 1) * BPC)
        eoffk = np.zeros((128, NG), dtype=np.float32)
        for g in range(NG):
            for r in range(GSZ):
                bk = k * BPC + (GSZ * g + r) // NCH
                eoffk[32 * r:32 * r + MEM, g] = qb[bk].astype(np.float32)
        in_maps.append({
            "feats": np.ascontiguousarray(f16[s]).reshape(T, 128, 4, NT),
            "wpt": wpt,
            "qat": np.ascontiguousarray(qat[:, s]),
            "qa4": np.ascontiguousarray(qa4[:, s]),
            "onesm": onesm,
            "eoff": eoffk,
        })

    trace = bool(int(os.environ.get("KERNEL_TRACE", "0")))
    tc_env = os.environ.get("KERNEL_TRACE_CORES", "")
    trace_cores = [int(x) for x in tc_env.split(",") if x] or None
    res = run_bass_kernel_spmd(nc, in_maps, core_ids=list(range(NCORES)),
                               trace=trace, trace_cores=trace_cores)
    LAST_RESULTS = res

    bpr = bp.astype(np.float32)                                  # [CODE]
    out = np.empty((B, 2 * CODE, HW), dtype=np.float32)
    for k in range(NCORES):
        xk = res.results[k]["out_x"]          # [128, T, 2, NT] int8
        uk = res.results[k]["out_u"]          # [128, T, 2, NT] fp8
        # sign-aware dequant: SDMA int8 cast truncates toward zero, so
        # the true value lies half an ulp beyond the integer
        xf = xk.astype(np.float32)
        xf += np.where(xk > 0, 0.5, np.where(xk < 0, -0.5, 0.0)).astype(np.float32)
        xf *= np.float32(1.0 / XSC)
        xq = (xf.reshape(128, BPC, NCH, 2, NT).transpose(1, 3, 0, 2, 4)
              .reshape(BPC, CODE, HW))
        xq += bpr[None, :, None]              # host-side projection bias
        uq = (uk.astype(np.float32).reshape(128, BPC, NCH, 2, NT)
              .transpose(1, 3, 0, 2, 4).reshape(BPC, CODE, HW))
        s = slice(k * BPC, (k + 1) * BPC)
        out[s, CODE:] = xq
        out[s, :CODE] = uq
    return out.reshape(B, 2 * CODE, H, W_SP)


if __name__ == "__main__":
    d = np.load("/tmp/inputs.npz")
    out = kernel(d["feats"], d["preds"], d["labels"], d["flag"], d["queue"], d["Wp"], d["bp"])
    exp = np.load("/tmp/expected.npy")
    err = np.abs(out - exp)
    print("absmax err:", err.max(), "scale-rel:", err.max() / np.abs(exp).max())
